# revision 1
# baseline (speedup 1.0000x reference)
"""DCNNv2 GNN message-passing kernel for 8 trn2 NeuronCores.

Strategy (memory-regime): shard external nodes (N=10000 -> 1250/core, padded
to 1280). The environment's device-side gather primitives are broken
(dma_gather ucode hangs the Q7; indirect_dma_start silently corrupts with
>1 offset column), so embedding-row gathers are materialized host-side into
per-core sequential streams; every model FLOP (neighbour sums, W/M/U/V
matmuls, relus, softmaxes, link MLP) runs on device in Bass across three
small NEFFs, with host-side shard exchange between phases:

  NEFF1: j-sum + s=relu(W e + M t), k-sum, softmax -> h shard
  NEFF2: ext-neighbour sum + relu(U h + V ext) + softmax -> e_all shard
  NEFF3: pair-concat MLP + leaky relu + 2-class softmax (as sigmoid of
         logit difference) -> probs
"""
import sys
sys.path.insert(0, "/opt/trn_rl_repo")
import numpy as np
import concourse.bacc as bacc
import concourse.mybir as mybir
from concourse.tile import TileContext
from concourse.masks import make_identity
from concourse.bass_utils import run_bass_kernel_spmd

F32 = mybir.dt.float32
AX = mybir.AxisListType = __import__("concourse.mybir", fromlist=["AxisListType"]).AxisListType
ALU = mybir.AluOpType
ACT = mybir.ActivationFunctionType

N, K, J, D, V, B = 10000, 16, 8, 128, 50000, 2048
NC_ = 8
NS = 1280              # padded nodes per core
NT = NS // 8           # 160 group tiles of 128 groups (8 nodes x 16 k)
NB = NS // 128         # 10 node blocks


def _softmax_block(nc, pool, blk_in, out_ap):
    """softmax along free dim of a [128,128] tile; writes to out_ap (sbuf)."""
    negmax = pool.tile([128, 1], F32, tag="negmax")
    nc.vector.tensor_reduce(out=negmax[:], in_=blk_in, axis=AX.X,
                            op=ALU.max, negate=True)
    ex = pool.tile([128, 128], F32, tag="ex")
    sm = pool.tile([128, 1], F32, tag="sm")
    nc.scalar.activation(out=ex[:], in_=blk_in, func=ACT.Exp,
                         bias=negmax[:], accum_out=sm[:])
    rec = pool.tile([128, 1], F32, tag="rec")
    nc.vector.reciprocal(rec[:], sm[:])
    nc.vector.tensor_scalar_mul(out_ap, ex[:], rec[:])


def _build_neff1():
    nc = bacc.Bacc("TRN2", target_bir_lowering=False, num_devices=NC_)
    nbrE = nc.dram_tensor("nbrE", [NT, 128, J * D], F32, kind="ExternalInput")
    embE = nc.dram_tensor("embE", [NT, 128, D], F32, kind="ExternalInput")
    WT = nc.dram_tensor("WT", [128, 128], F32, kind="ExternalInput")
    MT = nc.dram_tensor("MT", [128, 128], F32, kind="ExternalInput")
    hout = nc.dram_tensor("hout", [NB, 128, D], F32, kind="ExternalOutput")
    with TileContext(nc) as tc:
        with tc.tile_pool(name="w", bufs=1) as wpool, \
             tc.tile_pool(name="s", bufs=3) as pool, \
             tc.tile_pool(name="ps", bufs=2, space="PSUM") as psp:
            ident = wpool.tile([128, 128], F32)
            make_identity(nc, ident[:])
            wt = wpool.tile([128, 128], F32)
            mt = wpool.tile([128, 128], F32)
            nc.sync.dma_start(out=wt[:], in_=WT.ap())
            nc.sync.dma_start(out=mt[:], in_=MT.ap())
            R = wpool.tile([128, NS], F32)       # [f', node] accumulator
            nc.vector.memset(R[:], 0.0)
            for t in range(NT):
                nbr = pool.tile([128, J * D], F32, tag="nbr")
                nc.sync.dma_start(out=nbr[:], in_=nbrE[t])
                emb = pool.tile([128, D], F32, tag="emb")
                nc.sync.dma_start(out=emb[:], in_=embE[t])
                h4 = pool.tile([128, 4 * D], F32, tag="h4")
                nc.vector.tensor_tensor(out=h4[:], in0=nbr[:, 0:4 * D],
                                        in1=nbr[:, 4 * D:8 * D], op=ALU.add)
                h2 = pool.tile([128, 2 * D], F32, tag="h2")
                nc.vector.tensor_tensor(out=h2[:], in0=h4[:, 0:2 * D],
                                        in1=h4[:, 2 * D:4 * D], op=ALU.add)
                tsum = pool.tile([128, D], F32, tag="tsum")
                nc.vector.tensor_tensor(out=tsum[:], in0=h2[:, 0:D],
                                        in1=h2[:, D:2 * D], op=ALU.add)
                # transpose emb,tsum -> [f, grp]
                eT_p = psp.tile([128, 128], F32, tag="eT")
                nc.tensor.transpose(out=eT_p[:], in_=emb[:], identity=ident[:])
                eT = pool.tile([128, 128], F32, tag="eTs")
                nc.scalar.copy(eT[:], eT_p[:])
                tT_p = psp.tile([128, 128], F32, tag="tT")
                nc.tensor.transpose(out=tT_p[:], in_=tsum[:], identity=ident[:])
                tT = pool.tile([128, 128], F32, tag="tTs")
                nc.scalar.copy(tT[:], tT_p[:])
                acc = psp.tile([128, 128], F32, tag="acc")
                nc.tensor.matmul(out=acc[:], lhsT=wt[:], rhs=eT[:],
                                 start=True, stop=False)
                nc.tensor.matmul(out=acc[:], lhsT=mt[:], rhs=tT[:],
                                 start=False, stop=True)
                s = pool.tile([128, 128], F32, tag="s")
                nc.scalar.activation(out=s[:], in_=acc[:], func=ACT.Relu)
                # k-sum: cols g = n*16+k (8 nodes) -> [128, 8]
                k8 = pool.tile([128, 8 * 8], F32, tag="k8")
                sv = s[:].rearrange("p (n k) -> p n k", k=16)
                nc.vector.tensor_tensor(out=k8[:].rearrange("p (n k) -> p n k", k=8),
                                        in0=sv[:, :, 0:8], in1=sv[:, :, 8:16],
                                        op=ALU.add)
                k4 = pool.tile([128, 8 * 4], F32, tag="k4")
                k8v = k8[:].rearrange("p (n k) -> p n k", k=8)
                nc.vector.tensor_tensor(out=k4[:].rearrange("p (n k) -> p n k", k=4),
                                        in0=k8v[:, :, 0:4], in1=k8v[:, :, 4:8],
                                        op=ALU.add)
                k2 = pool.tile([128, 8 * 2], F32, tag="k2")
                k4v = k4[:].rearrange("p (n k) -> p n k", k=4)
                nc.vector.tensor_tensor(out=k2[:].rearrange("p (n k) -> p n k", k=2),
                                        in0=k4v[:, :, 0:2], in1=k4v[:, :, 2:4],
                                        op=ALU.add)
                k2v = k2[:].rearrange("p (n k) -> p n k", k=2)
                nc.vector.tensor_tensor(out=R[:, t * 8:(t + 1) * 8],
                                        in0=k2v[:, :, 0:1].rearrange("p n k -> p (n k)"),
                                        in1=k2v[:, :, 1:2].rearrange("p n k -> p (n k)"),
                                        op=ALU.add)
            # R [f', node] -> per 128-node block: transpose, softmax, out
            for b in range(NB):
                rT_p = psp.tile([128, 128], F32, tag="rT")
                nc.tensor.transpose(out=rT_p[:], in_=R[:, b * 128:(b + 1) * 128],
                                    identity=ident[:])
                rT = pool.tile([128, 128], F32, tag="rTs")
                nc.scalar.copy(rT[:], rT_p[:])
                hblk = pool.tile([128, 128], F32, tag="hblk")
                _softmax_block(nc, pool, rT[:], hblk[:])
                nc.sync.dma_start(out=hout[b], in_=hblk[:])
    nc.compile()
    return nc


def _build_neff2():
    nc = bacc.Bacc("TRN2", target_bir_lowering=False, num_devices=NC_)
    extE = nc.dram_tensor("extE", [NB, 128, 16 * D], F32, kind="ExternalInput")
    hOwn = nc.dram_tensor("hOwn", [NB, 128, D], F32, kind="ExternalInput")
    UT = nc.dram_tensor("UT", [128, 128], F32, kind="ExternalInput")
    VT = nc.dram_tensor("VT", [128, 128], F32, kind="ExternalInput")
    eout = nc.dram_tensor("eout", [NB, 128, D], F32, kind="ExternalOutput")
    with TileContext(nc) as tc:
        with tc.tile_pool(name="w", bufs=1) as wpool, \
             tc.tile_pool(name="s", bufs=3) as pool, \
             tc.tile_pool(name="ps", bufs=2, space="PSUM") as psp:
            ident = wpool.tile([128, 128], F32)
            make_identity(nc, ident[:])
            ut = wpool.tile([128, 128], F32)
            vt = wpool.tile([128, 128], F32)
            nc.sync.dma_start(out=ut[:], in_=UT.ap())
            nc.sync.dma_start(out=vt[:], in_=VT.ap())
            for b in range(NB):
                ext = pool.tile([128, 16 * D], F32, tag="ext")
                nc.sync.dma_start(out=ext[:], in_=extE[b])
                h = pool.tile([128, D], F32, tag="h")
                nc.sync.dma_start(out=h[:], in_=hOwn[b])
                e8 = pool.tile([128, 8 * D], F32, tag="e8")
                nc.vector.tensor_tensor(out=e8[:], in0=ext[:, 0:8 * D],
                                        in1=ext[:, 8 * D:16 * D], op=ALU.add)
                e4 = pool.tile([128, 4 * D], F32, tag="e4")
                nc.vector.tensor_tensor(out=e4[:], in0=e8[:, 0:4 * D],
                                        in1=e8[:, 4 * D:8 * D], op=ALU.add)
                e2 = pool.tile([128, 2 * D], F32, tag="e2")
                nc.vector.tensor_tensor(out=e2[:], in0=e4[:, 0:2 * D],
                                        in1=e4[:, 2 * D:4 * D], op=ALU.add)
                es = pool.tile([128, D], F32, tag="es")
                nc.vector.tensor_tensor(out=es[:], in0=e2[:, 0:D],
                                        in1=e2[:, D:2 * D], op=ALU.add)
                hT_p = psp.tile([128, 128], F32, tag="hT")
                nc.tensor.transpose(out=hT_p[:], in_=h[:], identity=ident[:])
                hT = pool.tile([128, 128], F32, tag="hTs")
                nc.scalar.copy(hT[:], hT_p[:])
                xT_p = psp.tile([128, 128], F32, tag="xT")
                nc.tensor.transpose(out=xT_p[:], in_=es[:], identity=ident[:])
                xT = pool.tile([128, 128], F32, tag="xTs")
                nc.scalar.copy(xT[:], xT_p[:])
                acc = psp.tile([128, 128], F32, tag="acc")
                nc.tensor.matmul(out=acc[:], lhsT=ut[:], rhs=hT[:],
                                 start=True, stop=False)
                nc.tensor.matmul(out=acc[:], lhsT=vt[:], rhs=xT[:],
                                 start=False, stop=True)
                pre = pool.tile([128, 128], F32, tag="pre")
                nc.scalar.activation(out=pre[:], in_=acc[:], func=ACT.Relu)
                # transpose back to [node, f]
                pT_p = psp.tile([128, 128], F32, tag="pT")
                nc.tensor.transpose(out=pT_p[:], in_=pre[:], identity=ident[:])
                pT = pool.tile([128, 128], F32, tag="pTs")
                nc.scalar.copy(pT[:], pT_p[:])
                eblk = pool.tile([128, 128], F32, tag="eblk")
                _softmax_block(nc, pool, pT[:], eblk[:])
                nc.sync.dma_start(out=eout[b], in_=eblk[:])
    nc.compile()
    return nc


def _build_neff3():
    nc = bacc.Bacc("TRN2", target_bir_lowering=False, num_devices=NC_)
    NP = B // NC_                   # 256 pairs per core
    ea = nc.dram_tensor("ea", [2, 128, D], F32, kind="ExternalInput")
    eb = nc.dram_tensor("eb", [2, 128, D], F32, kind="ExternalInput")
    W1aT = nc.dram_tensor("W1aT", [128, 128], F32, kind="ExternalInput")
    W1bT = nc.dram_tensor("W1bT", [128, 128], F32, kind="ExternalInput")
    b1t = nc.dram_tensor("b1t", [128, 1], F32, kind="ExternalInput")
    w2dT = nc.dram_tensor("w2dT", [128, 1], F32, kind="ExternalInput")
    b2d = nc.dram_tensor("b2d", [1, 1], F32, kind="ExternalInput")
    pout = nc.dram_tensor("pout", [2, NP], F32, kind="ExternalOutput")
    with TileContext(nc) as tc:
        with tc.tile_pool(name="w", bufs=1) as wpool, \
             tc.tile_pool(name="s", bufs=2) as pool, \
             tc.tile_pool(name="ps", bufs=2, space="PSUM") as psp:
            ident = wpool.tile([128, 128], F32)
            make_identity(nc, ident[:])
            w1a = wpool.tile([128, 128], F32)
            w1b = wpool.tile([128, 128], F32)
            b1s = wpool.tile([128, 1], F32)
            w2d = wpool.tile([128, 1], F32)
            b2s = wpool.tile([1, 1], F32)
            nc.sync.dma_start(out=w1a[:], in_=W1aT.ap())
            nc.sync.dma_start(out=w1b[:], in_=W1bT.ap())
            nc.sync.dma_start(out=b1s[:], in_=b1t.ap())
            nc.sync.dma_start(out=w2d[:], in_=w2dT.ap())
            nc.sync.dma_start(out=b2s[:], in_=b2d.ap())
            yac = psp.tile([128, NP], F32, tag="yac")
            for half in range(2):
                et = pool.tile([128, D], F32, tag="et")
                nc.sync.dma_start(out=et[:], in_=ea[half])
                eT_p = psp.tile([128, 128], F32, tag="eT")
                nc.tensor.transpose(out=eT_p[:], in_=et[:], identity=ident[:])
                eT = pool.tile([128, 128], F32, tag="eTs")
                nc.scalar.copy(eT[:], eT_p[:])
                nc.tensor.matmul(out=yac[:, half * 128:(half + 1) * 128],
                                 lhsT=w1a[:], rhs=eT[:], start=True, stop=False)
                bt = pool.tile([128, D], F32, tag="bt")
                nc.sync.dma_start(out=bt[:], in_=eb[half])
                bT_p = psp.tile([128, 128], F32, tag="bT")
                nc.tensor.transpose(out=bT_p[:], in_=bt[:], identity=ident[:])
                bT = pool.tile([128, 128], F32, tag="bTs")
                nc.scalar.copy(bT[:], bT_p[:])
                nc.tensor.matmul(out=yac[:, half * 128:(half + 1) * 128],
                                 lhsT=w1b[:], rhs=bT[:], start=False, stop=True)
            y0 = pool.tile([128, NP], F32, tag="y0")
            nc.scalar.activation(out=y0[:], in_=yac[:], func=ACT.Identity,
                                 bias=b1s[:])
            ys = pool.tile([128, NP], F32, tag="ys")
            nc.scalar.mul(ys[:], y0[:], 0.01)
            y = pool.tile([128, NP], F32, tag="y")
            nc.vector.tensor_tensor(out=y[:], in0=y0[:], in1=ys[:], op=ALU.max)
            dl = psp.tile([1, NP], F32, tag="dl")
            nc.tensor.matmul(out=dl[:], lhsT=w2d[:, 0:1], rhs=y[:],
                             start=True, stop=True)
            p0 = pool.tile([1, NP], F32, tag="p0")
            nc.scalar.activation(out=p0[:], in_=dl[:], func=ACT.Sigmoid,
                                 bias=b2s[:], scale=1.0)
            nb2 = pool.tile([1, 1], F32, tag="nb2")
            nc.scalar.mul(nb2[:], b2s[:], -1.0)
            p1 = pool.tile([1, NP], F32, tag="p1")
            nc.scalar.activation(out=p1[:], in_=dl[:], func=ACT.Sigmoid,
                                 bias=nb2[:], scale=-1.0)
            nc.sync.dma_start(out=pout[0:1], in_=p0[:])
            nc.sync.dma_start(out=pout[1:2], in_=p1[:])
    nc.compile()
    return nc


def kernel(batch, int_node_ids, int_neigh_ids, ext_neigh,
           E, W, M, U, V, W1, b1, W2, b2):
    batch = np.asarray(batch); int_node_ids = np.asarray(int_node_ids)
    int_neigh_ids = np.asarray(int_neigh_ids); ext_neigh = np.asarray(ext_neigh)
    E = np.asarray(E, np.float32)
    W = np.asarray(W, np.float32); M = np.asarray(M, np.float32)
    U = np.asarray(U, np.float32); V = np.asarray(V, np.float32)
    W1 = np.asarray(W1, np.float32); b1 = np.asarray(b1, np.float32)
    W2 = np.asarray(W2, np.float32); b2 = np.asarray(b2, np.float32)

    ids = int_node_ids.astype(np.int64)
    idsn = int_neigh_ids.astype(np.int64)
    ext = ext_neigh.astype(np.int64)
    bat = batch.astype(np.int64)

    # ---- Phase 1 inputs: per-core pre-gathered E rows, group-tile layout --
    in1, in2meta = [], []
    NSH = N // NC_                       # 1250 real nodes per core
    for c in range(NC_):
        lo = c * NSH
        idp = np.zeros((NS, K), np.int64)
        inp = np.zeros((NS, K, J), np.int64)
        idp[:NSH] = ids[lo:lo + NSH]
        inp[:NSH] = idsn[lo:lo + NSH]
        embE = E[idp].reshape(NT, 128, D)
        nbrE = E[inp.reshape(NS * K, J)].reshape(NT, 128, J * D)
        in1.append({"nbrE": nbrE, "embE": embE,
                    "WT": np.ascontiguousarray(W.T), "MT": np.ascontiguousarray(M.T)})
    nc1 = _build_neff1()
    res1 = run_bass_kernel_spmd(nc1, in1, core_ids=list(range(NC_)))
    h = np.zeros((N, D), np.float32)
    for c in range(NC_):
        hs = res1.results[c]["hout"].reshape(NS, D)
        h[c * NSH:(c + 1) * NSH] = hs[:NSH]

    # ---- Phase 2: host-gather h[ext_neigh] ------------------------------
    in2 = []
    for c in range(NC_):
        lo = c * NSH
        extp = np.zeros((NS, 16), np.int64)
        extp[:NSH] = ext[lo:lo + NSH]
        extE = h[extp].reshape(NB, 128, 16 * D)
        hOwn = np.zeros((NS, D), np.float32)
        hOwn[:NSH] = h[lo:lo + NSH]
        in2.append({"extE": extE, "hOwn": hOwn.reshape(NB, 128, D),
                    "UT": np.ascontiguousarray(U.T), "VT": np.ascontiguousarray(V.T)})
    nc2 = _build_neff2()
    res2 = run_bass_kernel_spmd(nc2, in2, core_ids=list(range(NC_)))
    e_all = np.zeros((N, D), np.float32)
    for c in range(NC_):
        es = res2.results[c]["eout"].reshape(NS, D)
        e_all[c * NSH:(c + 1) * NSH] = es[:NSH]

    # ---- Phase 3: link MLP ---------------------------------------------
    NP = B // NC_
    eaf = e_all[bat[:, 0]]
    ebf = e_all[bat[:, 1]]
    w2dv = (W2[0] - W2[1]).astype(np.float32).reshape(128, 1)
    b2dv = np.array([[b2[0] - b2[1]]], np.float32)
    in3 = []
    for c in range(NC_):
        sl = slice(c * NP, (c + 1) * NP)
        in3.append({
            "ea": eaf[sl].reshape(2, 128, D), "eb": ebf[sl].reshape(2, 128, D),
            "W1aT": np.ascontiguousarray(W1[:, :128].T),
            "W1bT": np.ascontiguousarray(W1[:, 128:].T),
            "b1t": b1.reshape(128, 1), "w2dT": w2dv, "b2d": b2dv})
    nc3 = _build_neff3()
    res3 = run_bass_kernel_spmd(nc3, in3, core_ids=list(range(NC_)))
    out = np.zeros((B, 2), np.float32)
    for c in range(NC_):
        p = res3.results[c]["pout"]          # [2, NP]
        out[c * NP:(c + 1) * NP, 0] = p[0]
        out[c * NP:(c + 1) * NP, 1] = p[1]
    return out

